# revision 27
# baseline (speedup 1.0000x reference)
"""Multi-head attention Trainium2 Bass kernel.

Problem: B=8, N=2048, C=768, H=12 heads, D=64 head dim.
  qkv = x @ w_qkv.T          -> [B, N, 3C]
  per head: softmax(q k^T / sqrt(D)) @ v
  y = attn_out @ w_proj.T + b_proj

Sharding: data parallel over batch — one batch element per NeuronCore (8 cores).

Per-core layout strategy (everything "transposed", feature-major):
  xT   [C, N]  — via PE transposes of x tiles
  qkvT [F, N] = W_qkv^T-stationary matmuls over xT   (F = 3C = 2304)
  S^T  [nk, nq] per head = kT-tile-stationary vs qT moving -> the softmax
       denominator comes from a ones-column appended to V in the A@V matmul
       (row 64 of the AV psum accumulates sum(exp(s))).
  exp via ScalarE (scale=1/8 folded in, no max subtraction: |scores| <~ 2.5)
  aT   [C, N] normalized attention output, fed as lhsT to the proj matmul.

Heads are processed in pairs occupying SBUF partitions 0-63 / 64-127 of
shared q/k/v tiles; their S^T matmuls use disjoint PE row groups and the
interleaving keeps ScalarE (the exp bottleneck, ~330us/core floor) saturated
while PE fills the gaps with the other head's matmuls.

All matmuls run in float32r (~1 cycle/row at free dim >= 256, rel err ~2e-4).
"""

import numpy as np

import concourse.bass as bass
import concourse.mybir as mybir
import concourse.tile as tile
from concourse import bacc
from concourse.bass_utils import run_bass_kernel_spmd
from concourse.masks import make_identity

B, N, C, H = 8, 2048, 768, 12
D = C // H            # 64
F = 3 * C             # 2304
NT = N // 128         # 16 seq tiles
CT = C // 128         # 6 channel tiles
FT = F // 128         # 18 qkv-feature tiles
NQ = 512              # query-chunk width (1 psum bank of fp32)
NCH = N // NQ         # 4 chunks
SCALE = float(D) ** -0.5

FP32 = mybir.dt.float32
FP32R = mybir.dt.float32r
EXP = mybir.ActivationFunctionType.Exp

_CACHED_NC = None


def _bc_ap(dram_ap, parts):
    """Partition-broadcast a 1-D DRAM AP to [parts, len] via stride-0."""
    return bass.AP(
        tensor=dram_ap.tensor,
        offset=dram_ap.offset,
        ap=[[0, parts]] + [list(p) for p in dram_ap.ap],
    )


def build():
    nc = bacc.Bacc()
    x = nc.dram_tensor("x", [N, C], FP32, kind="ExternalInput")
    w_qkv = nc.dram_tensor("w_qkv", [F, C], FP32, kind="ExternalInput")
    w_proj = nc.dram_tensor("w_proj", [C, C], FP32, kind="ExternalInput")
    b_proj = nc.dram_tensor("b_proj", [C], FP32, kind="ExternalInput")
    y = nc.dram_tensor("y", [N, C], FP32, kind="ExternalOutput")
    qkvT_d = nc.dram_tensor("qkvT_scratch", [F, N], FP32R)
    aT_d = nc.dram_tensor("aT_scratch", [C, N], FP32R)

    xr = x[:, :].bitcast(FP32R)
    wqr = w_qkv[:, :].bitcast(FP32R)
    wpr = w_proj[:, :].bitcast(FP32R)

    lp = nc.allow_low_precision("float32r psum accumulation is fp32-width")
    lp.__enter__()
    with tile.TileContext(nc) as tc:
        const_cm = tc.tile_pool(name="const", bufs=1)
        const = const_cm.__enter__()
        ident_f = const.tile([128, 128], FP32)
        make_identity(nc, ident_f)
        ident = const.tile([128, 128], FP32R)
        nc.vector.tensor_copy(ident, ident_f)
        ones_row_f = const.tile([1, D], FP32)
        nc.vector.memset(ones_row_f, 1.0)
        ones_row = const.tile([1, D], FP32R)
        nc.vector.tensor_copy(ones_row, ones_row_f)
        ones_col = const.tile([128, NT, 1], FP32)
        nc.vector.memset(ones_col, 1.0)
        bias_bc = const.tile([128, C], FP32)
        nc.gpsimd.dma_start(out=bias_bc, in_=_bc_ap(b_proj[:], 128))
        w_projT = const.tile([128, CT, C], FP32R)

        def load_transposed(stage, psum_t, src_r, rows, dst):
            """src_r: fp32r DRAM AP [rows*128, C]; dst[:, k, i*128...] gets
            the transpose of row-block i's c-tile k. Transposes grouped 4
            to a psum bank, evicted with one wide ScalarE copy."""
            for i0 in range(0, rows, 4):
                g = min(4, rows - i0)
                sts = []
                for i in range(i0, i0 + g):
                    st = stage.tile([128, C], FP32R, tag="stage")
                    nc.sync.dma_start(
                        out=st, in_=src_r[i * 128:(i + 1) * 128, :]
                    )
                    sts.append(st)
                for k in range(CT):
                    pt = psum_t.tile([128, 4, 128], FP32R, tag="pt")
                    for gi in range(g):
                        nc.tensor.transpose(
                            pt[:, gi, :],
                            sts[gi][:, k * 128:(k + 1) * 128],
                            ident,
                        )
                    nc.scalar.copy(
                        dst[:, k, i0 * 128:(i0 + g) * 128],
                        pt[:, 0:g, :],
                    )

        # ---------------- phase 0+1: transposes and qkv^T -> DRAM ----------
        with tc.tile_pool(name="ph1", bufs=1) as ph1, \
             tc.tile_pool(name="stage", bufs=10) as stage, \
             tc.tile_pool(name="evict1", bufs=3) as evict1, \
             tc.tile_pool(name="psum_t", bufs=2, space="PSUM") as psum_t, \
             tc.tile_pool(name="psum_q", bufs=4, space="PSUM") as psum_q:

            xT = ph1.tile([128, CT, N], FP32R)
            wqkvT = ph1.tile([128, CT, F], FP32R)

            load_transposed(stage, psum_t, wpr, CT, w_projT)
            load_transposed(stage, psum_t, xr, NT, xT)
            load_transposed(stage, psum_t, wqr, FT, wqkvT)

            # emit f-tiles in the order heads consume them: pair hp needs
            # {hp, 6+hp, 12+hp}
            m_order = []
            for hp in range(CT):
                m_order += [hp, CT + hp, 2 * CT + hp]
            for m in m_order:
                ev = evict1.tile([128, N], FP32R, tag="ev")
                for j in range(NCH):
                    ps = psum_q.tile([128, NQ], FP32, tag="psq")
                    for k in range(CT):
                        nc.tensor.matmul(
                            ps,
                            wqkvT[:, k, m * 128:(m + 1) * 128],
                            xT[:, k, j * NQ:(j + 1) * NQ],
                            start=(k == 0),
                            stop=(k == CT - 1),
                        )
                    nc.vector.tensor_copy(ev[:, j * NQ:(j + 1) * NQ], ps)
                nc.sync.dma_start(
                    out=qkvT_d[m * 128:(m + 1) * 128, :], in_=ev
                )

        # ---------------- phase 2: attention, head pairs --------------------
        with tc.tile_pool(name="hpool", bufs=2) as hpool, \
             tc.tile_pool(name="spool", bufs=1) as spool, \
             tc.tile_pool(name="small", bufs=2) as small, \
             tc.tile_pool(name="psum_s", bufs=2, space="PSUM") as psum_s, \
             tc.tile_pool(name="psum_av", bufs=2, space="PSUM") as psum_av:

            for hp in range(H // 2):
                qTt = hpool.tile([128, N], FP32R, tag="qT")
                nc.sync.dma_start(
                    out=qTt, in_=qkvT_d[hp * 128:(hp + 1) * 128, :]
                )
                kTt = hpool.tile([128, N], FP32R, tag="kT")
                nc.sync.dma_start(
                    out=kTt, in_=qkvT_d[C + hp * 128:C + (hp + 1) * 128, :]
                )
                vTt = hpool.tile([128, N], FP32R, tag="vT")
                nc.sync.dma_start(
                    out=vTt, in_=qkvT_d[2 * C + hp * 128:2 * C + (hp + 1) * 128, :]
                )
                vaugs = []
                for a in range(2):
                    vaug = hpool.tile([128, NT, D + 1], FP32R, tag=f"vaug{a}")
                    nc.vector.tensor_copy(vaug[:, :, D:D + 1], ones_col)
                    lo = a * D
                    for t0 in range(0, NT, 8):
                        pt = psum_av.tile([128, 8, D], FP32R, tag="av")
                        for g in range(8):
                            t = t0 + g
                            nc.tensor.transpose(
                                pt[:, g, :],
                                vTt[lo:lo + D, t * 128:(t + 1) * 128],
                                ident[lo:lo + D, lo:lo + D],
                            )
                        nc.vector.tensor_copy(
                            vaug[:, t0:t0 + 8, 0:D], pt
                        )
                    vaugs.append(vaug)

                # nk-tile group sizes: 3-bank psum tiles double-buffered so
                # ScalarE exp(g) overlaps the S^T matmuls of g+1.
                GROUPS = (3, 3, 3, 3, 2, 2)
                for j in range(NCH):
                    expSs = []
                    for a in range(2):
                        expS = spool.tile(
                            [128, NT, NQ], FP32R,
                            tag=f"expS{a}", name=f"expS{a}",
                        )
                        lo = a * D
                        t = 0
                        for gsz in GROUPS:
                            sps = psum_s.tile([128, 3, NQ], FP32, tag="sps")
                            for u in range(gsz):
                                nc.tensor.matmul(
                                    sps[:, u, :],
                                    kTt[lo:lo + D, (t + u) * 128:(t + u + 1) * 128],
                                    qTt[lo:lo + D, j * NQ:(j + 1) * NQ],
                                    start=True,
                                    stop=True,
                                )
                            nc.scalar.activation(
                                out=expS[:, t:t + gsz, :],
                                in_=sps[:, 0:gsz, :],
                                func=EXP,
                                scale=SCALE,
                            )
                            t += gsz
                        expSs.append(expS)
                    for a in range(2):
                        h = 2 * hp + a
                        av = psum_av.tile([D + 1, NQ], FP32, tag="av")
                        for t in range(NT):
                            nc.tensor.matmul(
                                av,
                                vaugs[a][:, t, :],
                                expSs[a][:, t, :],
                                start=(t == 0),
                                stop=(t == NT - 1),
                            )
                        recip = small.tile([1, NQ], FP32R, tag="recip")
                        nc.vector.reciprocal(recip, av[D:D + 1, :])
                        bc = psum_av.tile([D, NQ], FP32, tag="av")
                        nc.tensor.matmul(
                            bc, ones_row, recip, start=True, stop=True
                        )
                        bc_sb = small.tile([D, NQ], FP32, tag="bc_sb")
                        nc.vector.tensor_copy(bc_sb, bc)
                        aTt = small.tile([D, NQ], FP32R, tag="aT_sb")
                        nc.vector.tensor_mul(aTt, av[0:D, :], bc_sb)
                        nc.sync.dma_start(
                            out=aT_d[h * D:(h + 1) * D, j * NQ:(j + 1) * NQ],
                            in_=aTt,
                        )

        # ---------------- phase 3: output projection ------------------------
        with tc.tile_pool(name="ppool", bufs=3) as ppool, \
             tc.tile_pool(name="psum_p", bufs=2, space="PSUM") as psum_p:
            NO = 384
            for i in range(NT):
                a_sb = ppool.tile([128, CT, 128], FP32R, tag="a_sb")
                nc.sync.dma_start(
                    out=a_sb,
                    in_=aT_d[:, i * 128:(i + 1) * 128].rearrange(
                        "(ko p) n -> p ko n", p=128
                    ),
                )
                for half in range(2):
                    ps = psum_p.tile([128, NO], FP32, tag="psp")
                    for k in range(CT):
                        nc.tensor.matmul(
                            ps,
                            a_sb[:, k, :],
                            w_projT[:, k, half * NO:(half + 1) * NO],
                            start=(k == 0),
                            stop=(k == CT - 1),
                        )
                    y_sb = ppool.tile([128, NO], FP32, tag="y_sb")
                    nc.vector.tensor_add(
                        y_sb, ps, bias_bc[:, half * NO:(half + 1) * NO]
                    )
                    nc.sync.dma_start(
                        out=y[i * 128:(i + 1) * 128, half * NO:(half + 1) * NO],
                        in_=y_sb,
                    )
        const_cm.__exit__(None, None, None)
    lp.__exit__(None, None, None)

    nc.finalize()
    return nc


def get_nc():
    global _CACHED_NC
    if _CACHED_NC is None:
        _CACHED_NC = build()
    return _CACHED_NC


LAST_RESULT = None


def kernel(x, w_qkv, w_proj, b_proj, **run_kwargs):
    x = np.ascontiguousarray(np.asarray(x, dtype=np.float32))
    w_qkv = np.ascontiguousarray(np.asarray(w_qkv, dtype=np.float32))
    w_proj = np.ascontiguousarray(np.asarray(w_proj, dtype=np.float32))
    b_proj = np.ascontiguousarray(np.asarray(b_proj, dtype=np.float32))
    assert x.shape == (B, N, C)

    nc = get_nc()
    in_maps = [
        {"x": x[i], "w_qkv": w_qkv, "w_proj": w_proj, "b_proj": b_proj}
        for i in range(B)
    ]
    res = run_bass_kernel_spmd(nc, in_maps, list(range(B)), **run_kwargs)
    global LAST_RESULT
    LAST_RESULT = res
    out = np.stack([res.results[i]["y"] for i in range(B)], axis=0)
    return out


if __name__ == "__main__":
    rng = np.random.default_rng(0)
    x = rng.standard_normal((B, N, C), dtype=np.float32)
    w_qkv = (rng.standard_normal((F, C)) * 0.02).astype(np.float32)
    w_proj = (rng.standard_normal((C, C)) * 0.02).astype(np.float32)
    b_proj = (rng.standard_normal((C,)) * 0.02).astype(np.float32)
    out = kernel(x=x, w_qkv=w_qkv, w_proj=w_proj, b_proj=b_proj)
    print("out", out.shape, out.dtype, float(np.abs(out).max()))
